# revision 18
# baseline (speedup 1.0000x reference)
"""Trainium2 Bass kernel for nn_DPFABase (DPFA knowledge-tracing attention).

Full-input contract: kernel(**inputs) takes the unsharded inputs and returns
the full [B, S] float32 output. Internally: data-parallel over batch across
8 NeuronCores (16 examples per core); the [V, H] embedding table is
replicated (uploaded bf16), beta/response tables are folded into small
per-example host-marshaled tensors.

Per-core pipeline (no table prepass):
  Per example e (16 per core):
    1. One 1024-idx dma_gather pulls the raw bf16 embedding rows for
       [512 hist | 512 next] tokens -> G [128, 8, 128] (token t at
       partition t%128, chunk t//128).
    2. Row sumsq per chunk (next chunks on DVE, hist chunks on ACT) ->
       all-DVE Quake rsqrt (bit-trick seed + 2 Newton steps) gives
       1/|row|. Next chunks (4..7) are normalized into a separate tile;
       hist chunks stay raw -- their norm is applied later inside the
       exp as a per-partition scale.
    3. G is bounced through DRAM and read back with dma_start_transpose,
       yielding TT [128(H), 1024] = [hist_T | next_T] with no PE work.
    4. QK matmuls (causal-blocked) -> scoresT [s, q] in PSUM; ACT exp with
       scale=1/|h_s| (hist norm) and per-partition bias (-k*s + centering;
       the per-q part of time decay cancels in softmax); causal mask on
       the diagonal tile; num/den matmuls against host-provided
       [mastery*pad | pad] -> [q, 2] PSUM.
  Finals: ability = num/den, sigmoid(ability - beta_next), PE transpose,
  one DMA to the [16, 512] output.
"""
import numpy as np

B, S, H, V = 128, 512, 128, 10000
NCORES = 8
EXC = B // NCORES          # examples per core = 16
VPAD = 10112               # 79 * 128
NTMP = 6                   # DRAM bounce buffers

_CACHE = {}


def _build_nc():
    import concourse.bacc as bacc
    import concourse.mybir as mybir
    from concourse.tile import TileContext

    f32 = mybir.dt.float32
    bf16 = mybir.dt.bfloat16
    i16 = mybir.dt.int16
    i32 = mybir.dt.int32
    AF = mybir.ActivationFunctionType
    ALU = mybir.AluOpType

    nc = bacc.Bacc()

    emb = nc.declare_dram_parameter("emb", [VPAD, H], bf16, isOutput=False)
    idx = nc.declare_dram_parameter("idx", [128, EXC * 64], i16, isOutput=False)
    taux = nc.declare_dram_parameter("taux", [128, EXC * 8], bf16, isOutput=False)
    bnext = nc.declare_dram_parameter("bnext", [128, EXC * 4], f32, isOutput=False)
    biaspp = nc.declare_dram_parameter("biaspp", [128, 4], f32, isOutput=False)
    causal = nc.declare_dram_parameter("causal", [128, 128], bf16, isOutput=False)
    identf = nc.declare_dram_parameter("identf", [128, 128], f32, isOutput=False)
    out = nc.declare_dram_parameter("out", [EXC, S], f32, isOutput=True)

    tmp = nc.dram_tensor("tmp", [NTMP, 1024, H], bf16)

    with TileContext(nc) as tc:
        with (
            tc.tile_pool(name="persist", bufs=1) as persist,
            tc.tile_pool(name="main", bufs=3) as main,
            tc.tile_pool(name="tts", bufs=2) as tts,
            tc.tile_pool(name="ejs", bufs=2) as ejs,
            tc.tile_pool(name="psC", bufs=2, space="PSUM") as psC,
            tc.tile_pool(name="psD", bufs=2, space="PSUM") as psD,
        ):
            # ---------- constants ----------
            idx_t = persist.tile([128, EXC * 64], i16, name="idx_t")
            nc.sync.dma_start(out=idx_t[:], in_=idx[:, :])
            bias_t = persist.tile([128, 4], f32, name="bias_t")
            nc.sync.dma_start(out=bias_t[:], in_=biaspp[:, :])
            causal_t = persist.tile([128, 128], bf16, name="causal_t")
            nc.sync.dma_start(out=causal_t[:], in_=causal[:, :])
            identf_t = persist.tile([128, 128], f32, name="identf_t")
            nc.sync.dma_start(out=identf_t[:], in_=identf[:, :])
            taux_t = persist.tile([128, EXC * 8], bf16, name="taux_t")
            nc.sync.dma_start(out=taux_t[:], in_=taux[:, :])
            bnext_t = persist.tile([128, EXC * 4], f32, name="bnext_t")
            nc.sync.dma_start(out=bnext_t[:], in_=bnext[:, :])
            F_all = persist.tile([128, 8 * EXC], f32, name="F_all")

            # ---------- software-pipelined main loop ----------
            # Stage A(e): gather, sumsq, rsqrt, normalize-next, store, transpose
            # Stage B(e): QK, exp, causal, num/den
            # Emission: A(0..LEAD-1), then B(e), A(e+LEAD) interleaved — keeps
            # early-stage entries ahead of late-stage waits in every engine
            # queue so no queue head blocks upstream work.
            LEAD = 3
            rn_tiles = {}

            def rsqrt_dve(rn, ss, t1, t2):
                """rn = 1/sqrt(ss), all-DVE: Quake seed + 2 Newton steps."""
                nc.vector.tensor_scalar(
                    out=t1.bitcast(i32), in0=ss.bitcast(i32), scalar1=1,
                    scalar2=None, op0=ALU.arith_shift_right,
                )
                nc.vector.tensor_scalar(
                    out=t2.bitcast(i32), in0=t1.bitcast(i32), scalar1=-1,
                    scalar2=None, op0=ALU.bitwise_xor,
                )
                nc.vector.tensor_scalar(
                    out=rn.bitcast(i32), in0=t2.bitcast(i32), scalar1=0x5F3759E0,
                    scalar2=None, op0=ALU.add,
                )
                for _ in range(1):
                    nc.vector.tensor_tensor(out=t1, in0=rn, in1=rn, op=ALU.mult)
                    nc.vector.scalar_tensor_tensor(
                        t2, t1, -0.5, ss, op0=ALU.mult, op1=ALU.mult
                    )
                    nc.vector.scalar_tensor_tensor(
                        rn, t2, 1.5, rn, op0=ALU.add, op1=ALU.mult
                    )

            def stage_a(e):
                G = main.tile([128, 8, H], bf16, name="G", tag="G", bufs=6)
                nc.gpsimd.dma_gather(
                    G[:], emb[:, :], idx_t[:, 64 * e:64 * e + 64],
                    1024, 1024, H, elem_step=H,
                )
                ss = main.tile([128, 8], f32, name="ss", tag="ss", bufs=3)
                dump = main.tile([128, H], bf16, name="dump", tag="dump", bufs=2)
                dumpA = main.tile([128, H], bf16, name="dumpA", tag="dumpA", bufs=2)
                # next-chunk sumsq on DVE (critical path to the store)
                for c in range(4, 8):
                    nc.vector.scalar_tensor_tensor(
                        dump[:], G[:, c, :], 1.0, G[:, c, :],
                        op0=ALU.mult, op1=ALU.mult, accum_out=ss[:, c:c + 1],
                    )
                # hist-chunk sumsq on ACT (only feeds exp scale in stage B)
                for c in range(4):
                    nc.scalar.activation(
                        dumpA[:], G[:, c, :], AF.Square, accum_out=ss[:, c:c + 1]
                    )
                rn = main.tile([128, 8], f32, name="rn", tag="rn", bufs=LEAD + 2)
                t1 = main.tile([128, 8], f32, name="t1", tag="t1", bufs=2)
                t2 = main.tile([128, 8], f32, name="t2", tag="t2", bufs=2)
                rsqrt_dve(rn[:], ss[:], t1[:], t2[:])
                # normalize next chunks into Gn (separate tile: no in-place RMW)
                Gn = main.tile([128, 4, H], bf16, name="Gn", tag="Gn", bufs=3)
                for c in range(4, 8):
                    nc.scalar.activation(
                        Gn[:, c - 4, :], G[:, c, :], AF.Copy, scale=rn[:, c:c + 1]
                    )
                rn_tiles[e] = rn
                # bounce through DRAM to transpose: tmp[(c t), h] = G[t, c, h]
                te = tmp[e % NTMP]
                nc.sync.dma_start(
                    out=te[:, :].rearrange("(c t) h -> t c h", c=8)[:, 0:4, :],
                    in_=G[:, 0:4, :],
                )
                nc.sync.dma_start(
                    out=te[:, :].rearrange("(c t) h -> t c h", c=8)[:, 4:8, :],
                    in_=Gn[:],
                )
                TT = tts.tile([128, 1024], bf16, name="TT", tag="TT", bufs=4)
                nc.sync.dma_start_transpose(out=TT[:], in_=te[:, :])
                return TT

            def stage_b(e, TT):
                rn = rn_tiles.pop(e)
                e_tiles = []
                for j in range(4):
                    n_j = 512 - 128 * j
                    sc = psC.tile([128, 512], f32, name="sc", tag=f"sc{j % 2}", bufs=2)
                    nc.tensor.matmul(
                        sc[:, 0:n_j],
                        TT[:, 128 * j:128 * (j + 1)],
                        TT[:, 512 + 128 * j:1024],
                        start=True, stop=True,
                    )
                    e_j = ejs.tile([128, 512], bf16, name="e_j", tag=f"e_j{j}", bufs=3)
                    nc.scalar.activation(
                        e_j[:, 0:n_j], sc[:, 0:n_j], AF.Exp,
                        bias=bias_t[:, j:j + 1], scale=rn[:, j:j + 1],
                    )
                    nc.vector.tensor_tensor(
                        out=e_j[:, 0:128], in0=e_j[:, 0:128], in1=causal_t[:],
                        op=ALU.mult,
                    )
                    e_tiles.append(e_j)

                # num/den matmuls: out[q-block c] accumulates over j<=c
                nd = psD.tile([128, 8], f32, name="nd", tag="nd", bufs=2)
                for c in range(4):
                    for j in range(c + 1):
                        nc.tensor.matmul(
                            nd[:, 2 * c:2 * c + 2],
                            e_tiles[j][:, 128 * (c - j):128 * (c - j + 1)],
                            taux_t[:, 8 * e + 2 * j:8 * e + 2 * j + 2],
                            start=(j == 0), stop=(j == c),
                        )
                nc.vector.tensor_copy(F_all[:, 8 * e:8 * e + 8], nd[:])

            tt_tiles = {}
            for e in range(LEAD):
                tt_tiles[e] = stage_a(e)
            for e in range(EXC):
                stage_b(e, tt_tiles.pop(e))
                if e + LEAD < EXC:
                    tt_tiles[e + LEAD] = stage_a(e + LEAD)

            # ---------- finals ----------
            F3 = F_all[:].rearrange("p (x t) -> p x t", t=2)
            rd = persist.tile([128, 64], f32, name="rd")
            nc.vector.reciprocal(rd[:], F3[:, :, 1])
            at = persist.tile([128, 64], f32, name="at")
            nc.vector.tensor_tensor(out=at[:], in0=F3[:, :, 0], in1=rd[:], op=ALU.mult)
            zt = persist.tile([128, 64], f32, name="zt")
            nc.vector.tensor_tensor(out=zt[:], in0=at[:], in1=bnext_t[:], op=ALU.subtract)
            ot = persist.tile([128, 64], f32, name="ot")
            nc.scalar.activation(ot[:], zt[:], AF.Sigmoid)
            pso = psC.tile([128, 128], f32, name="pso", tag="pso", bufs=1)
            nc.tensor.transpose(pso[0:64, :], ot[:], identf_t[:])
            otr = persist.tile([64, 128], f32, name="otr")
            nc.vector.tensor_copy(otr[:], pso[0:64, :])
            nc.sync.dma_start(
                out=out[:, :].rearrange("e (x q) -> (e x) q", x=4), in_=otr[:]
            )

    nc.finalize()
    return nc


def _marshal(inputs):
    import ml_dtypes

    bf16 = ml_dtypes.bfloat16
    hist = np.asarray(inputs["history_items"]).astype(np.int64)
    nxt = np.asarray(inputs["next_items"]).astype(np.int64)
    corrects = np.asarray(inputs["history_corrects"]).astype(np.int64)
    E = np.asarray(inputs["item_embedding"], dtype=np.float32)
    beta = np.asarray(inputs["item_beta_weights"], dtype=np.float32)
    resp = np.asarray(inputs["item_response_vals"], dtype=np.float32)
    k = float(np.asarray(inputs["td_kernel"]).reshape(-1)[0])

    emb_pad = np.ones((VPAD, H), dtype=np.float32)
    emb_pad[:V] = E
    emb16 = emb_pad.astype(bf16)

    p = np.arange(128, dtype=np.float32)
    biaspp = np.stack(
        [-k * (128.0 * j + p) + k * (S / 2 - 0.5) for j in range(4)], axis=1
    ).astype(np.float32)
    causal = (p[:, None] <= p[None, :]).astype(bf16)  # keep s<=q within tile
    identf = np.eye(128, dtype=np.float32)

    # per-example tables
    is_c = (corrects == 2).astype(np.int64)
    mastery = resp[hist, is_c]                       # [B, S]
    pad = (hist != 0).astype(np.float32)             # [B, S]
    mp = (mastery * pad).astype(np.float32)
    bn_full = beta[nxt]                              # [B, S]

    in_maps = []
    for core in range(NCORES):
        idx_c = np.zeros((128, EXC * 64), dtype=np.int16)
        taux_c = np.zeros((128, EXC * 8), dtype=np.float32)
        bnext_c = np.zeros((128, EXC * 4), dtype=np.float32)
        for e in range(EXC):
            b = core * EXC + e
            ids = np.concatenate([hist[b], nxt[b]]).astype(np.int16)
            w = ids.reshape(64, 16).T  # [16, 64]: token t -> part t%16, col t//16
            for g in range(8):
                idx_c[16 * g:16 * (g + 1), 64 * e:64 * e + 64] = w
            mp_b = mp[b].reshape(4, 128).T           # [128(p), 4(j)]
            pad_b = pad[b].reshape(4, 128).T
            for j in range(4):
                taux_c[:, 8 * e + 2 * j] = mp_b[:, j]
                taux_c[:, 8 * e + 2 * j + 1] = pad_b[:, j]
            bnext_c[:, 4 * e:4 * e + 4] = bn_full[b].reshape(4, 128).T
        in_maps.append(
            dict(
                emb=emb16,
                idx=idx_c,
                taux=taux_c.astype(bf16),
                bnext=bnext_c,
                biaspp=biaspp,
                causal=causal,
                identf=identf,
            )
        )
    return in_maps


def kernel(**inputs) -> np.ndarray:
    from concourse.bass_utils import run_bass_kernel_spmd

    if "nc" not in _CACHE:
        _CACHE["nc"] = _build_nc()
    nc = _CACHE["nc"]
    in_maps = _marshal(inputs)
    res = run_bass_kernel_spmd(nc, in_maps, list(range(NCORES))).results
    out = np.concatenate([res[c]["out"] for c in range(NCORES)], axis=0)
    return np.ascontiguousarray(out).astype(np.float32)


# revision 19
# speedup vs baseline: 1.0335x; 1.0335x over previous
"""Trainium2 Bass kernel for nn_DPFABase (DPFA knowledge-tracing attention).

Full-input contract: kernel(**inputs) takes the unsharded inputs and returns
the full [B, S] float32 output. Internally: data-parallel over batch across
8 NeuronCores (16 examples per core); the [V, H] embedding table is
replicated (uploaded bf16), beta/response tables are folded into small
per-example host-marshaled tensors.

Per-core pipeline (no table prepass):
  Per example e (16 per core):
    1. One 1024-idx dma_gather pulls the raw bf16 embedding rows for
       [512 hist | 512 next] tokens -> G [128, 8, 128] (token t at
       partition t%128, chunk t//128).
    2. Row sumsq per chunk (next chunks on DVE, hist chunks on ACT) ->
       all-DVE Quake rsqrt (bit-trick seed + 2 Newton steps) gives
       1/|row|. Next chunks (4..7) are normalized into a separate tile;
       hist chunks stay raw -- their norm is applied later inside the
       exp as a per-partition scale.
    3. G is bounced through DRAM and read back with dma_start_transpose,
       yielding TT [128(H), 1024] = [hist_T | next_T] with no PE work.
    4. QK matmuls (causal-blocked) -> scoresT [s, q] in PSUM; ACT exp with
       scale=1/|h_s| (hist norm) and per-partition bias (-k*s + centering;
       the per-q part of time decay cancels in softmax); causal mask on
       the diagonal tile; num/den matmuls against host-provided
       [mastery*pad | pad] -> [q, 2] PSUM.
  Finals: ability = num/den, sigmoid(ability - beta_next), PE transpose,
  one DMA to the [16, 512] output.
"""
import numpy as np

B, S, H, V = 128, 512, 128, 10000
NCORES = 8
EXC = B // NCORES          # examples per core = 16
VPAD = 10112               # 79 * 128
NTMP = 6                   # DRAM bounce buffers

_CACHE = {}


def _build_nc():
    import concourse.bacc as bacc
    import concourse.mybir as mybir
    from concourse.tile import TileContext

    f32 = mybir.dt.float32
    bf16 = mybir.dt.bfloat16
    i16 = mybir.dt.int16
    i32 = mybir.dt.int32
    AF = mybir.ActivationFunctionType
    ALU = mybir.AluOpType

    nc = bacc.Bacc()

    emb = nc.declare_dram_parameter("emb", [VPAD, H], bf16, isOutput=False)
    idx = nc.declare_dram_parameter("idx", [128, EXC * 64], i16, isOutput=False)
    taux = nc.declare_dram_parameter("taux", [128, EXC * 8], bf16, isOutput=False)
    bnext = nc.declare_dram_parameter("bnext", [128, EXC * 4], f32, isOutput=False)
    biaspp = nc.declare_dram_parameter("biaspp", [128, 4], f32, isOutput=False)
    causal = nc.declare_dram_parameter("causal", [128, 128], bf16, isOutput=False)
    identf = nc.declare_dram_parameter("identf", [128, 128], f32, isOutput=False)
    out = nc.declare_dram_parameter("out", [EXC, S], f32, isOutput=True)

    tmp = nc.dram_tensor("tmp", [NTMP, 1024, H], bf16)

    with TileContext(nc) as tc:
        with (
            tc.tile_pool(name="persist", bufs=1) as persist,
            tc.tile_pool(name="main", bufs=3) as main,
            tc.tile_pool(name="tts", bufs=2) as tts,
            tc.tile_pool(name="ejs", bufs=2) as ejs,
            tc.tile_pool(name="psC", bufs=2, space="PSUM") as psC,
            tc.tile_pool(name="psD", bufs=2, space="PSUM") as psD,
        ):
            # ---------- constants ----------
            idx_t = persist.tile([128, EXC * 64], i16, name="idx_t")
            nc.sync.dma_start(out=idx_t[:], in_=idx[:, :])
            bias_t = persist.tile([128, 4], f32, name="bias_t")
            nc.sync.dma_start(out=bias_t[:], in_=biaspp[:, :])
            causal_t = persist.tile([128, 128], bf16, name="causal_t")
            nc.sync.dma_start(out=causal_t[:], in_=causal[:, :])
            identf_t = persist.tile([128, 128], f32, name="identf_t")
            nc.sync.dma_start(out=identf_t[:], in_=identf[:, :])
            taux_t = persist.tile([128, EXC * 8], bf16, name="taux_t")
            nc.sync.dma_start(out=taux_t[:], in_=taux[:, :])
            bnext_t = persist.tile([128, EXC * 4], f32, name="bnext_t")
            nc.sync.dma_start(out=bnext_t[:], in_=bnext[:, :])
            F_all = persist.tile([128, 8 * EXC], f32, name="F_all")

            # ---------- software-pipelined main loop ----------
            # Stage A(e): gather, sumsq, rsqrt, normalize-next, store, transpose
            # Stage B(e): QK, exp, causal, num/den
            # Emission: A(0..LEAD-1), then B(e), A(e+LEAD) interleaved — keeps
            # early-stage entries ahead of late-stage waits in every engine
            # queue so no queue head blocks upstream work.
            LEAD = 3
            rn_tiles = {}

            def rsqrt_dve(rn, ss, t1, t2):
                """rn = 1/sqrt(ss), all-DVE: Quake seed + 2 Newton steps."""
                nc.vector.tensor_scalar(
                    out=t1.bitcast(i32), in0=ss.bitcast(i32), scalar1=1,
                    scalar2=None, op0=ALU.arith_shift_right,
                )
                nc.vector.tensor_scalar(
                    out=t2.bitcast(i32), in0=t1.bitcast(i32), scalar1=-1,
                    scalar2=None, op0=ALU.bitwise_xor,
                )
                nc.vector.tensor_scalar(
                    out=rn.bitcast(i32), in0=t2.bitcast(i32), scalar1=0x5F3759E0,
                    scalar2=None, op0=ALU.add,
                )
                for _ in range(1):
                    nc.vector.tensor_tensor(out=t1, in0=rn, in1=rn, op=ALU.mult)
                    nc.vector.scalar_tensor_tensor(
                        t2, t1, -0.5, ss, op0=ALU.mult, op1=ALU.mult
                    )
                    nc.vector.scalar_tensor_tensor(
                        rn, t2, 1.5, rn, op0=ALU.add, op1=ALU.mult
                    )

            def stage_a(e):
                G = main.tile([128, 8, H], bf16, name="G", tag="G", bufs=6)
                nc.gpsimd.dma_gather(
                    G[:], emb[:, :], idx_t[:, 64 * e:64 * e + 64],
                    1024, 1024, H, elem_step=H,
                )
                ss = main.tile([128, 8], f32, name="ss", tag="ss", bufs=3)
                dump = main.tile([128, H], bf16, name="dump", tag="dump", bufs=2)
                dumpA = main.tile([128, H], bf16, name="dumpA", tag="dumpA", bufs=2)
                # next-chunk sumsq on DVE (critical path to the store)
                for c in range(4, 8):
                    nc.vector.scalar_tensor_tensor(
                        dump[:], G[:, c, :], 1.0, G[:, c, :],
                        op0=ALU.mult, op1=ALU.mult, accum_out=ss[:, c:c + 1],
                    )
                # hist-chunk sumsq on ACT (only feeds exp scale in stage B)
                for c in range(4):
                    nc.scalar.activation(
                        dumpA[:], G[:, c, :], AF.Square, accum_out=ss[:, c:c + 1]
                    )
                rn = main.tile([128, 8], f32, name="rn", tag="rn", bufs=LEAD + 2)
                t1 = main.tile([128, 8], f32, name="t1", tag="t1", bufs=2)
                t2 = main.tile([128, 8], f32, name="t2", tag="t2", bufs=2)
                rsqrt_dve(rn[:], ss[:], t1[:], t2[:])
                # normalize next chunks into Gn (separate tile: no in-place RMW)
                Gn = main.tile([128, 4, H], bf16, name="Gn", tag="Gn", bufs=3)
                for c in range(4, 8):
                    nc.vector.tensor_scalar_mul(
                        Gn[:, c - 4, :], G[:, c, :], rn[:, c:c + 1]
                    )
                rn_tiles[e] = rn
                # bounce through DRAM to transpose: tmp[(c t), h] = G[t, c, h]
                te = tmp[e % NTMP]
                nc.sync.dma_start(
                    out=te[:, :].rearrange("(c t) h -> t c h", c=8)[:, 0:4, :],
                    in_=G[:, 0:4, :],
                )
                nc.sync.dma_start(
                    out=te[:, :].rearrange("(c t) h -> t c h", c=8)[:, 4:8, :],
                    in_=Gn[:],
                )
                TT = tts.tile([128, 1024], bf16, name="TT", tag="TT", bufs=4)
                nc.sync.dma_start_transpose(out=TT[:], in_=te[:, :])
                return TT

            def stage_b(e, TT):
                rn = rn_tiles.pop(e)
                e_tiles = []
                for j in range(4):
                    n_j = 512 - 128 * j
                    sc = psC.tile([128, 512], f32, name="sc", tag=f"sc{j % 2}", bufs=2)
                    nc.tensor.matmul(
                        sc[:, 0:n_j],
                        TT[:, 128 * j:128 * (j + 1)],
                        TT[:, 512 + 128 * j:1024],
                        start=True, stop=True,
                    )
                    e_j = ejs.tile([128, 512], bf16, name="e_j", tag=f"e_j{j}", bufs=3)
                    nc.scalar.activation(
                        e_j[:, 0:n_j], sc[:, 0:n_j], AF.Exp,
                        bias=bias_t[:, j:j + 1], scale=rn[:, j:j + 1],
                    )
                    nc.vector.tensor_tensor(
                        out=e_j[:, 0:128], in0=e_j[:, 0:128], in1=causal_t[:],
                        op=ALU.mult,
                    )
                    e_tiles.append(e_j)

                # num/den matmuls: out[q-block c] accumulates over j<=c
                nd = psD.tile([128, 8], f32, name="nd", tag="nd", bufs=2)
                for c in range(4):
                    for j in range(c + 1):
                        nc.tensor.matmul(
                            nd[:, 2 * c:2 * c + 2],
                            e_tiles[j][:, 128 * (c - j):128 * (c - j + 1)],
                            taux_t[:, 8 * e + 2 * j:8 * e + 2 * j + 2],
                            start=(j == 0), stop=(j == c),
                        )
                nc.vector.tensor_copy(F_all[:, 8 * e:8 * e + 8], nd[:])

            tt_tiles = {}
            for e in range(LEAD):
                tt_tiles[e] = stage_a(e)
            for e in range(EXC):
                stage_b(e, tt_tiles.pop(e))
                if e + LEAD < EXC:
                    tt_tiles[e + LEAD] = stage_a(e + LEAD)

            # ---------- finals ----------
            F3 = F_all[:].rearrange("p (x t) -> p x t", t=2)
            rd = persist.tile([128, 64], f32, name="rd")
            nc.vector.reciprocal(rd[:], F3[:, :, 1])
            at = persist.tile([128, 64], f32, name="at")
            nc.vector.tensor_tensor(out=at[:], in0=F3[:, :, 0], in1=rd[:], op=ALU.mult)
            zt = persist.tile([128, 64], f32, name="zt")
            nc.vector.tensor_tensor(out=zt[:], in0=at[:], in1=bnext_t[:], op=ALU.subtract)
            ot = persist.tile([128, 64], f32, name="ot")
            nc.scalar.activation(ot[:], zt[:], AF.Sigmoid)
            pso = psC.tile([128, 128], f32, name="pso", tag="pso", bufs=1)
            nc.tensor.transpose(pso[0:64, :], ot[:], identf_t[:])
            otr = persist.tile([64, 128], f32, name="otr")
            nc.vector.tensor_copy(otr[:], pso[0:64, :])
            nc.sync.dma_start(
                out=out[:, :].rearrange("e (x q) -> (e x) q", x=4), in_=otr[:]
            )

    nc.finalize()
    return nc


def _marshal(inputs):
    import ml_dtypes

    bf16 = ml_dtypes.bfloat16
    hist = np.asarray(inputs["history_items"]).astype(np.int64)
    nxt = np.asarray(inputs["next_items"]).astype(np.int64)
    corrects = np.asarray(inputs["history_corrects"]).astype(np.int64)
    E = np.asarray(inputs["item_embedding"], dtype=np.float32)
    beta = np.asarray(inputs["item_beta_weights"], dtype=np.float32)
    resp = np.asarray(inputs["item_response_vals"], dtype=np.float32)
    k = float(np.asarray(inputs["td_kernel"]).reshape(-1)[0])

    emb_pad = np.ones((VPAD, H), dtype=np.float32)
    emb_pad[:V] = E
    emb16 = emb_pad.astype(bf16)

    p = np.arange(128, dtype=np.float32)
    biaspp = np.stack(
        [-k * (128.0 * j + p) + k * (S / 2 - 0.5) for j in range(4)], axis=1
    ).astype(np.float32)
    causal = (p[:, None] <= p[None, :]).astype(bf16)  # keep s<=q within tile
    identf = np.eye(128, dtype=np.float32)

    # per-example tables
    is_c = (corrects == 2).astype(np.int64)
    mastery = resp[hist, is_c]                       # [B, S]
    pad = (hist != 0).astype(np.float32)             # [B, S]
    mp = (mastery * pad).astype(np.float32)
    bn_full = beta[nxt]                              # [B, S]

    in_maps = []
    for core in range(NCORES):
        idx_c = np.zeros((128, EXC * 64), dtype=np.int16)
        taux_c = np.zeros((128, EXC * 8), dtype=np.float32)
        bnext_c = np.zeros((128, EXC * 4), dtype=np.float32)
        for e in range(EXC):
            b = core * EXC + e
            ids = np.concatenate([hist[b], nxt[b]]).astype(np.int16)
            w = ids.reshape(64, 16).T  # [16, 64]: token t -> part t%16, col t//16
            for g in range(8):
                idx_c[16 * g:16 * (g + 1), 64 * e:64 * e + 64] = w
            mp_b = mp[b].reshape(4, 128).T           # [128(p), 4(j)]
            pad_b = pad[b].reshape(4, 128).T
            for j in range(4):
                taux_c[:, 8 * e + 2 * j] = mp_b[:, j]
                taux_c[:, 8 * e + 2 * j + 1] = pad_b[:, j]
            bnext_c[:, 4 * e:4 * e + 4] = bn_full[b].reshape(4, 128).T
        in_maps.append(
            dict(
                emb=emb16,
                idx=idx_c,
                taux=taux_c.astype(bf16),
                bnext=bnext_c,
                biaspp=biaspp,
                causal=causal,
                identf=identf,
            )
        )
    return in_maps


def kernel(**inputs) -> np.ndarray:
    from concourse.bass_utils import run_bass_kernel_spmd

    if "nc" not in _CACHE:
        _CACHE["nc"] = _build_nc()
    nc = _CACHE["nc"]
    in_maps = _marshal(inputs)
    res = run_bass_kernel_spmd(nc, in_maps, list(range(NCORES))).results
    out = np.concatenate([res[c]["out"] for c in range(NCORES)], axis=0)
    return np.ascontiguousarray(out).astype(np.float32)


# revision 20
# speedup vs baseline: 1.0554x; 1.0212x over previous
"""Trainium2 Bass kernel for nn_DPFABase (DPFA knowledge-tracing attention).

Full-input contract: kernel(**inputs) takes the unsharded inputs and returns
the full [B, S] float32 output. Internally: data-parallel over batch across
8 NeuronCores (16 examples per core); the [V, H] embedding table is
replicated (uploaded bf16), beta/response tables are folded into small
per-example host-marshaled tensors.

Per-core pipeline (no table prepass):
  Per example e (16 per core):
    1. One 1024-idx dma_gather pulls the raw bf16 embedding rows for
       [512 hist | 512 next] tokens -> G [128, 8, 128] (token t at
       partition t%128, chunk t//128).
    2. Row sumsq per chunk (next chunks on DVE, hist chunks on ACT) ->
       all-DVE Quake rsqrt (bit-trick seed + 2 Newton steps) gives
       1/|row|. Next chunks (4..7) are normalized into a separate tile;
       hist chunks stay raw -- their norm is applied later inside the
       exp as a per-partition scale.
    3. G is bounced through DRAM and read back with dma_start_transpose,
       yielding TT [128(H), 1024] = [hist_T | next_T] with no PE work.
    4. QK matmuls (causal-blocked) -> scoresT [s, q] in PSUM; ACT exp with
       scale=1/|h_s| (hist norm) and per-partition bias (-k*s + centering;
       the per-q part of time decay cancels in softmax); causal mask on
       the diagonal tile; num/den matmuls against host-provided
       [mastery*pad | pad] -> [q, 2] PSUM.
  Finals: ability = num/den, sigmoid(ability - beta_next), PE transpose,
  one DMA to the [16, 512] output.
"""
import numpy as np

B, S, H, V = 128, 512, 128, 10000
NCORES = 8
EXC = B // NCORES          # examples per core = 16
VPAD = 10112               # 79 * 128
NTMP = 6                   # DRAM bounce buffers

_CACHE = {}


def _build_nc():
    import concourse.bacc as bacc
    import concourse.mybir as mybir
    from concourse.tile import TileContext

    f32 = mybir.dt.float32
    bf16 = mybir.dt.bfloat16
    i16 = mybir.dt.int16
    i32 = mybir.dt.int32
    AF = mybir.ActivationFunctionType
    ALU = mybir.AluOpType

    nc = bacc.Bacc()

    emb = nc.declare_dram_parameter("emb", [VPAD, H], bf16, isOutput=False)
    idx = nc.declare_dram_parameter("idx", [128, EXC * 64], i16, isOutput=False)
    taux = nc.declare_dram_parameter("taux", [128, EXC * 8], bf16, isOutput=False)
    bnext = nc.declare_dram_parameter("bnext", [128, EXC * 4], f32, isOutput=False)
    biaspp = nc.declare_dram_parameter("biaspp", [128, 4], f32, isOutput=False)
    causal = nc.declare_dram_parameter("causal", [128, 128], bf16, isOutput=False)
    identf = nc.declare_dram_parameter("identf", [128, 128], f32, isOutput=False)
    out = nc.declare_dram_parameter("out", [EXC, S], f32, isOutput=True)

    tmp = nc.dram_tensor("tmp", [NTMP, 1024, H], bf16)

    with TileContext(nc) as tc:
        with (
            tc.tile_pool(name="persist", bufs=1) as persist,
            tc.tile_pool(name="main", bufs=3) as main,
            tc.tile_pool(name="tts", bufs=2) as tts,
            tc.tile_pool(name="ejs", bufs=2) as ejs,
            tc.tile_pool(name="psC", bufs=2, space="PSUM") as psC,
            tc.tile_pool(name="psD", bufs=2, space="PSUM") as psD,
        ):
            # ---------- constants ----------
            idx_t = persist.tile([128, EXC * 64], i16, name="idx_t")
            nc.sync.dma_start(out=idx_t[:], in_=idx[:, :])
            bias_t = persist.tile([128, 4], f32, name="bias_t")
            nc.sync.dma_start(out=bias_t[:], in_=biaspp[:, :])
            causal_t = persist.tile([128, 128], bf16, name="causal_t")
            nc.sync.dma_start(out=causal_t[:], in_=causal[:, :])
            identf_t = persist.tile([128, 128], f32, name="identf_t")
            nc.sync.dma_start(out=identf_t[:], in_=identf[:, :])
            taux_t = persist.tile([128, EXC * 8], bf16, name="taux_t")
            nc.sync.dma_start(out=taux_t[:], in_=taux[:, :])
            bnext_t = persist.tile([128, EXC * 4], f32, name="bnext_t")
            nc.sync.dma_start(out=bnext_t[:], in_=bnext[:, :])
            F_all = persist.tile([128, 8 * EXC], f32, name="F_all")

            # ---------- software-pipelined main loop ----------
            # Stage A(e): gather, sumsq, rsqrt, normalize-next, store, transpose
            # Stage B(e): QK, exp, causal, num/den
            # Emission: A(0..LEAD-1), then B(e), A(e+LEAD) interleaved — keeps
            # early-stage entries ahead of late-stage waits in every engine
            # queue so no queue head blocks upstream work.
            LEAD = 3
            rn_tiles = {}

            def rsqrt_dve(rn, ss, t1, t2):
                """rn = 1/sqrt(ss), all-DVE: Quake seed + 2 Newton steps."""
                nc.vector.tensor_scalar(
                    out=t1.bitcast(i32), in0=ss.bitcast(i32), scalar1=1,
                    scalar2=None, op0=ALU.arith_shift_right,
                )
                nc.vector.tensor_scalar(
                    out=t2.bitcast(i32), in0=t1.bitcast(i32), scalar1=-1,
                    scalar2=None, op0=ALU.bitwise_xor,
                )
                nc.vector.tensor_scalar(
                    out=rn.bitcast(i32), in0=t2.bitcast(i32), scalar1=0x5F3759E0,
                    scalar2=None, op0=ALU.add,
                )
                for _ in range(2):
                    nc.vector.tensor_tensor(out=t1, in0=rn, in1=rn, op=ALU.mult)
                    nc.vector.scalar_tensor_tensor(
                        t2, t1, -0.5, ss, op0=ALU.mult, op1=ALU.mult
                    )
                    nc.vector.scalar_tensor_tensor(
                        rn, t2, 1.5, rn, op0=ALU.add, op1=ALU.mult
                    )

            def stage_a(e):
                G = main.tile([128, 8, H], bf16, name="G", tag="G", bufs=6)
                nc.gpsimd.dma_gather(
                    G[:], emb[:, :], idx_t[:, 64 * e:64 * e + 64],
                    1024, 1024, H, elem_step=H,
                )
                ss = main.tile([128, 8], f32, name="ss", tag="ss", bufs=3)
                dump = main.tile([128, H], bf16, name="dump", tag="dump", bufs=2)
                dumpA = main.tile([128, H], bf16, name="dumpA", tag="dumpA", bufs=2)
                # next-chunk sumsq on DVE (critical path to the store)
                for c in range(4, 8):
                    nc.vector.scalar_tensor_tensor(
                        dump[:], G[:, c, :], 1.0, G[:, c, :],
                        op0=ALU.mult, op1=ALU.mult, accum_out=ss[:, c:c + 1],
                    )
                # hist-chunk sumsq on ACT (only feeds exp scale in stage B)
                for c in range(4):
                    nc.scalar.activation(
                        dumpA[:], G[:, c, :], AF.Square, accum_out=ss[:, c:c + 1]
                    )
                rn = main.tile([128, 8], f32, name="rn", tag="rn", bufs=LEAD + 2)
                t1 = main.tile([128, 8], f32, name="t1", tag="t1", bufs=2)
                t2 = main.tile([128, 8], f32, name="t2", tag="t2", bufs=2)
                rsqrt_dve(rn[:], ss[:], t1[:], t2[:])
                # normalize next chunks into Gn (separate tile: no in-place RMW)
                Gn = main.tile([128, 4, H], bf16, name="Gn", tag="Gn", bufs=3)
                for c in range(4, 8):
                    nc.vector.tensor_scalar_mul(
                        Gn[:, c - 4, :], G[:, c, :], rn[:, c:c + 1]
                    )
                rn_tiles[e] = rn
                # bounce through DRAM to transpose: tmp[(c t), h] = G[t, c, h]
                te = tmp[e % NTMP]
                nc.sync.dma_start(
                    out=te[:, :].rearrange("(c t) h -> t c h", c=8)[:, 0:4, :],
                    in_=G[:, 0:4, :],
                )
                nc.sync.dma_start(
                    out=te[:, :].rearrange("(c t) h -> t c h", c=8)[:, 4:8, :],
                    in_=Gn[:],
                )
                TT = tts.tile([128, 1024], bf16, name="TT", tag="TT", bufs=4)
                nc.sync.dma_start_transpose(out=TT[:], in_=te[:, :])
                return TT

            def stage_b(e, TT):
                rn = rn_tiles.pop(e)
                e_tiles = []
                for j in range(4):
                    n_j = 512 - 128 * j
                    sc = psC.tile([128, 512], f32, name="sc", tag=f"sc{j % 2}", bufs=2)
                    nc.tensor.matmul(
                        sc[:, 0:n_j],
                        TT[:, 128 * j:128 * (j + 1)],
                        TT[:, 512 + 128 * j:1024],
                        start=True, stop=True,
                    )
                    e_j = ejs.tile([128, 512], bf16, name="e_j", tag=f"e_j{j}", bufs=3)
                    nc.scalar.activation(
                        e_j[:, 0:n_j], sc[:, 0:n_j], AF.Exp,
                        bias=bias_t[:, j:j + 1], scale=rn[:, j:j + 1],
                    )
                    nc.vector.tensor_tensor(
                        out=e_j[:, 0:128], in0=e_j[:, 0:128], in1=causal_t[:],
                        op=ALU.mult,
                    )
                    e_tiles.append(e_j)

                # num/den matmuls: out[q-block c] accumulates over j<=c
                nd = psD.tile([128, 8], f32, name="nd", tag="nd", bufs=2)
                for c in range(4):
                    for j in range(c + 1):
                        nc.tensor.matmul(
                            nd[:, 2 * c:2 * c + 2],
                            e_tiles[j][:, 128 * (c - j):128 * (c - j + 1)],
                            taux_t[:, 8 * e + 2 * j:8 * e + 2 * j + 2],
                            start=(j == 0), stop=(j == c),
                        )
                nc.vector.tensor_copy(F_all[:, 8 * e:8 * e + 8], nd[:])

            tt_tiles = {}
            for e in range(LEAD):
                tt_tiles[e] = stage_a(e)
            for e in range(EXC):
                stage_b(e, tt_tiles.pop(e))
                if e + LEAD < EXC:
                    tt_tiles[e + LEAD] = stage_a(e + LEAD)

            # ---------- finals ----------
            F3 = F_all[:].rearrange("p (x t) -> p x t", t=2)
            rd = persist.tile([128, 64], f32, name="rd")
            nc.vector.reciprocal(rd[:], F3[:, :, 1])
            at = persist.tile([128, 64], f32, name="at")
            nc.vector.tensor_tensor(out=at[:], in0=F3[:, :, 0], in1=rd[:], op=ALU.mult)
            zt = persist.tile([128, 64], f32, name="zt")
            nc.vector.tensor_tensor(out=zt[:], in0=at[:], in1=bnext_t[:], op=ALU.subtract)
            ot = persist.tile([128, 64], f32, name="ot")
            nc.scalar.activation(ot[:], zt[:], AF.Sigmoid)
            pso = psC.tile([128, 128], f32, name="pso", tag="pso", bufs=1)
            nc.tensor.transpose(pso[0:64, :], ot[:], identf_t[:])
            otr = persist.tile([64, 128], f32, name="otr")
            nc.vector.tensor_copy(otr[:], pso[0:64, :])
            nc.sync.dma_start(
                out=out[:, :].rearrange("e (x q) -> (e x) q", x=4), in_=otr[:]
            )

    nc.finalize()
    return nc


def _marshal(inputs):
    import ml_dtypes

    bf16 = ml_dtypes.bfloat16
    hist = np.asarray(inputs["history_items"]).astype(np.int64)
    nxt = np.asarray(inputs["next_items"]).astype(np.int64)
    corrects = np.asarray(inputs["history_corrects"]).astype(np.int64)
    E = np.asarray(inputs["item_embedding"], dtype=np.float32)
    beta = np.asarray(inputs["item_beta_weights"], dtype=np.float32)
    resp = np.asarray(inputs["item_response_vals"], dtype=np.float32)
    k = float(np.asarray(inputs["td_kernel"]).reshape(-1)[0])

    emb_pad = np.ones((VPAD, H), dtype=np.float32)
    emb_pad[:V] = E
    emb16 = emb_pad.astype(bf16)

    p = np.arange(128, dtype=np.float32)
    biaspp = np.stack(
        [-k * (128.0 * j + p) + k * (S / 2 - 0.5) for j in range(4)], axis=1
    ).astype(np.float32)
    causal = (p[:, None] <= p[None, :]).astype(bf16)  # keep s<=q within tile
    identf = np.eye(128, dtype=np.float32)

    # per-example tables
    is_c = (corrects == 2).astype(np.int64)
    mastery = resp[hist, is_c]                       # [B, S]
    pad = (hist != 0).astype(np.float32)             # [B, S]
    mp = (mastery * pad).astype(np.float32)
    bn_full = beta[nxt]                              # [B, S]

    in_maps = []
    for core in range(NCORES):
        idx_c = np.zeros((128, EXC * 64), dtype=np.int16)
        taux_c = np.zeros((128, EXC * 8), dtype=np.float32)
        bnext_c = np.zeros((128, EXC * 4), dtype=np.float32)
        for e in range(EXC):
            b = core * EXC + e
            ids = np.concatenate([hist[b], nxt[b]]).astype(np.int16)
            w = ids.reshape(64, 16).T  # [16, 64]: token t -> part t%16, col t//16
            for g in range(8):
                idx_c[16 * g:16 * (g + 1), 64 * e:64 * e + 64] = w
            mp_b = mp[b].reshape(4, 128).T           # [128(p), 4(j)]
            pad_b = pad[b].reshape(4, 128).T
            for j in range(4):
                taux_c[:, 8 * e + 2 * j] = mp_b[:, j]
                taux_c[:, 8 * e + 2 * j + 1] = pad_b[:, j]
            bnext_c[:, 4 * e:4 * e + 4] = bn_full[b].reshape(4, 128).T
        in_maps.append(
            dict(
                emb=emb16,
                idx=idx_c,
                taux=taux_c.astype(bf16),
                bnext=bnext_c,
                biaspp=biaspp,
                causal=causal,
                identf=identf,
            )
        )
    return in_maps


def kernel(**inputs) -> np.ndarray:
    from concourse.bass_utils import run_bass_kernel_spmd

    if "nc" not in _CACHE:
        _CACHE["nc"] = _build_nc()
    nc = _CACHE["nc"]
    in_maps = _marshal(inputs)
    res = run_bass_kernel_spmd(nc, in_maps, list(range(NCORES))).results
    out = np.concatenate([res[c]["out"] for c in range(NCORES)], axis=0)
    return np.ascontiguousarray(out).astype(np.float32)
